# revision 1
# baseline (speedup 1.0000x reference)
"""NetVLAD layer on 8 Trainium2 NeuronCores (Bass/Tile).

Problem: descriptors [B=16, D=512, N=4096] f32, W [K=64, D], b [K],
centers [D, K].
  scores = softmax_K(W @ desc + b)            [B, K, N]
  agg[b,d,k] = sum_n scores[b,k,n] desc[b,d,n]
  vlad = agg - centers * sum_n(scores);  intra-L2-norm over D; global L2.

Sharding: data-parallel over B across 8 cores (2 items per core);
W/b/centers replicated.

Per-core kernel (per item):
  - cast-load desc -> SBUF bf16 [D, N] (SWDGE cast DMA)
  - mm1: scores[K, 512-chunk] = sum_t WT_tile[t].T @ desc[t], psum f32
  - ACT: exp_s = Exp(scores + b) -> bf16 SBUF (bias fused, per-partition)
  - PE transpose exp_s 128-col chunks -> expT [n128, K] psum
  - DVE softmax: Z = rowsum(expT); softT = expT * (1/Z) -> bf16
  - PE transpose desc 128x128 tiles -> descT [n128, D] (bf16), DVE evac
  - mm2: aggT[K, D] += softT_c.T @ descT_c  (contract n, 32 chunks)
         ssum[K, 1] += softT_c.T @ ones
  - tail: vladT = (-centers.T * ssum) + aggT; intra-norm over D (free dim);
          global norm via ones-matmul partition reduction; DMA out [K, D]
Host side: slice/concat over B, transpose [K, D] -> [D, K] flatten.
"""

import sys

sys.path.insert(0, "/opt/trn_rl_repo")

import numpy as np
import ml_dtypes

B, D, K, N = 16, 512, 64, 4096
N_CORES = 8
B_PER = B // N_CORES           # 2 items per core
DT = D // 128                  # 4 d-tiles
NC128 = N // 128               # 32 n-chunks of 128
NC512 = N // 512               # 8 n-chunks of 512

_CACHE = {}


def _build(stage=5):
    import concourse.bass as bass  # noqa: F401
    import concourse.tile as tile
    from concourse import bacc, mybir
    from contextlib import ExitStack

    bf16 = mybir.dt.bfloat16
    f32 = mybir.dt.float32
    AF = mybir.ActivationFunctionType
    OP = mybir.AluOpType
    AX = mybir.AxisListType

    nc = bacc.Bacc("TRN2", target_bir_lowering=False, debug=False,
                   num_devices=N_CORES)

    desc_d = nc.dram_tensor("desc", [B_PER, D, N], f32, kind="ExternalInput").ap()
    wt_d = nc.dram_tensor("wt", [DT, 128, K], bf16, kind="ExternalInput").ap()
    b_d = nc.dram_tensor("bias", [K, 1], f32, kind="ExternalInput").ap()
    cneg_d = nc.dram_tensor("cneg", [K, D], f32, kind="ExternalInput").ap()
    eye128_d = nc.dram_tensor("eye128", [128, 128], bf16, kind="ExternalInput").ap()
    eye64_d = nc.dram_tensor("eye64", [64, 64], bf16, kind="ExternalInput").ap()
    onesb_d = nc.dram_tensor("onesb", [128, 1], bf16, kind="ExternalInput").ap()
    onesf_d = nc.dram_tensor("onesf", [64, 64], f32, kind="ExternalInput").ap()
    out_d = nc.dram_tensor("out", [B_PER, K, D], f32, kind="ExternalOutput").ap()

    with tile.TileContext(nc) as tc, ExitStack() as ctx:
        const = ctx.enter_context(tc.tile_pool(name="const", bufs=1))
        big = ctx.enter_context(tc.tile_pool(name="big", bufs=2))
        med = ctx.enter_context(tc.tile_pool(name="med", bufs=2))
        small = ctx.enter_context(tc.tile_pool(name="small", bufs=4))
        ps_sc = ctx.enter_context(tc.tile_pool(name="ps_sc", bufs=2, space="PSUM"))
        ps_scT = ctx.enter_context(tc.tile_pool(name="ps_scT", bufs=2, space="PSUM"))
        ps_dT = ctx.enter_context(tc.tile_pool(name="ps_dT", bufs=2, space="PSUM"))
        ps_agg = ctx.enter_context(tc.tile_pool(name="ps_agg", bufs=1, space="PSUM"))
        ps_tiny = ctx.enter_context(tc.tile_pool(name="ps_tiny", bufs=1, space="PSUM"))

        # ---- constants ----
        wt_sb = const.tile([128, DT, K], bf16, tag="wt")
        for t in range(DT):
            nc.sync.dma_start(out=wt_sb[:, t, :], in_=wt_d[t])
        b_sb = const.tile([K, 1], f32, tag="b")
        nc.sync.dma_start(out=b_sb[:], in_=b_d[:])
        cneg_sb = const.tile([K, D], f32, tag="cneg")
        nc.sync.dma_start(out=cneg_sb[:], in_=cneg_d[:])
        eye128_sb = const.tile([128, 128], bf16, tag="eye128")
        nc.sync.dma_start(out=eye128_sb[:], in_=eye128_d[:])
        eye64_sb = const.tile([64, 64], bf16, tag="eye64")
        nc.sync.dma_start(out=eye64_sb[:], in_=eye64_d[:])
        onesb_sb = const.tile([128, 1], bf16, tag="onesb")
        nc.sync.dma_start(out=onesb_sb[:], in_=onesb_d[:])
        onesf_sb = const.tile([64, 64], f32, tag="onesf")
        nc.sync.dma_start(out=onesf_sb[:], in_=onesf_d[:])
        eps_sb = const.tile([K, 1], f32, tag="eps")
        nc.vector.memset(eps_sb[:], 1e-24)

        for i in range(B_PER):
            # ---- load descriptors (cast f32 -> bf16) ----
            desc_bf = big.tile([128, DT, N], bf16, tag="desc_bf")
            for ns in range(4):
                nsl = slice(1024 * ns, 1024 * (ns + 1))
                for t in range(DT):
                    nc.gpsimd.dma_start(
                        out=desc_bf[:, t, nsl],
                        in_=desc_d[i, 128 * t:128 * (t + 1), nsl],
                    )
            descT = big.tile([128, NC128, D], bf16, tag="descT")
            exp_s = med.tile([K, N], bf16, tag="exp_s")
            softT = med.tile([128, NC128, K], bf16, tag="softT")
            agg_ps = ps_agg.tile([K, D], f32, tag="agg")
            tiny_ps = ps_tiny.tile([K, 4], f32, tag="tiny")

            for c8 in range(NC512):
                csl = slice(512 * c8, 512 * (c8 + 1))
                # mm1: scores chunk [K, 512]
                sc_ps = ps_sc.tile([K, 512], f32, tag="sc")
                for t in range(DT):
                    nc.tensor.matmul(
                        sc_ps[:], lhsT=wt_sb[:, t, :], rhs=desc_bf[:, t, csl],
                        start=(t == 0), stop=(t == DT - 1),
                    )
                # exp(scores + b) -> bf16
                nc.scalar.activation(out=exp_s[:, csl], in_=sc_ps[:],
                                     func=AF.Exp, bias=b_sb[:], scale=1.0)
                if stage < 2:
                    continue
                # transpose scores chunks to [n128, K]
                scT_ps = ps_scT.tile([128, 4, K], bf16, tag="scT")
                for j in range(4):
                    c = 4 * c8 + j
                    nc.tensor.transpose(
                        scT_ps[:, j, :], exp_s[:, 128 * c:128 * (c + 1)],
                        eye64_sb[:],
                    )
                for j in range(4):
                    c = 4 * c8 + j
                    z_sb = small.tile([128, 1], f32, tag="z")
                    nc.vector.reduce_sum(z_sb[:], scT_ps[:, j, :], axis=AX.X)
                    r_sb = small.tile([128, 1], f32, tag="r")
                    nc.vector.reciprocal(r_sb[:], z_sb[:])
                    nc.vector.tensor_scalar_mul(softT[:, c, :], scT_ps[:, j, :],
                                                r_sb[:])
            # debug-stage truncations: still produce an output so nothing
            # is dead-code-eliminated
            if stage < 2:
                nc.gpsimd.dma_start(out=out_d[i], in_=exp_s[0:64, 0:512])
                continue
            if stage < 3:
                nc.gpsimd.dma_start(out=out_d[i], in_=softT[0:64, 0:8, :])
                continue
            # desc transposes + mm2
            for c in range(NC128):
                dT_ps = ps_dT.tile([128, DT, 128], bf16, tag="dT")
                for t in range(DT):
                    nc.tensor.transpose(
                        dT_ps[:, t, :], desc_bf[:, t, 128 * c:128 * (c + 1)],
                        eye128_sb[:],
                    )
                if stage == 3.1:
                    continue  # PE transposes only, no evac
                if stage == 3.2:
                    nc.scalar.copy(descT[:, c, :], dT_ps[:, :, :])
                elif stage == 3.3:
                    for t in range(DT):
                        nc.vector.tensor_copy(descT[:, c, 128 * t:128 * (t + 1)],
                                              dT_ps[:, t, :])
                else:
                    nc.vector.tensor_copy(descT[:, c, :], dT_ps[:, :, :])
                if stage < 4:
                    continue
                if stage != 4.2:
                    nc.tensor.matmul(agg_ps[:], lhsT=softT[:, c, :],
                                     rhs=descT[:, c, :],
                                     start=(c == 0), stop=(c == NC128 - 1))
                if stage != 4.1:
                    nc.tensor.matmul(tiny_ps[:, 0:1], lhsT=softT[:, c, :],
                                     rhs=onesb_sb[:],
                                     start=(c == 0), stop=(c == NC128 - 1))
            if stage < 4:
                if stage == 3.1:
                    nc.gpsimd.dma_start(out=out_d[i], in_=exp_s[0:64, 0:512])
                else:
                    nc.gpsimd.dma_start(out=out_d[i], in_=descT[0:64, 0, :])
                continue
            if stage < 5:
                if stage == 4.2:
                    nc.gpsimd.dma_start(out=out_d[i], in_=descT[0:64, 0, :])
                else:
                    agg_sb = med.tile([K, D], f32, tag="agg_sb")
                    nc.scalar.copy(agg_sb[:], agg_ps[:])
                    nc.sync.dma_start(out=out_d[i], in_=agg_sb[:])
                continue

            # ---- tail: vlad + normalizations ----
            ssum_sb = small.tile([K, 1], f32, tag="ssum")
            nc.scalar.copy(ssum_sb[:], tiny_ps[:, 0:1])
            vlad_sb = med.tile([K, D], f32, tag="vlad")
            nc.vector.scalar_tensor_tensor(
                vlad_sb[:], in0=cneg_sb[:], scalar=ssum_sb[:], in1=agg_ps[:],
                op0=OP.mult, op1=OP.add,
            )
            if stage == 5.1:
                nc.sync.dma_start(out=out_d[i], in_=vlad_sb[:])
                continue
            # NOTE: tensor_tensor_reduce crashes TRN2 here (device
            # unrecoverable) -- use separate mul + reduce instead.
            sq_sb = med.tile([K, D], f32, tag="sq")
            ss_sb = small.tile([K, 1], f32, tag="ss")
            nc.vector.tensor_mul(sq_sb[:], vlad_sb[:], vlad_sb[:])
            nc.vector.reduce_sum(ss_sb[:], sq_sb[:], axis=AX.X)
            sn_sb = small.tile([K, 1], f32, tag="sn")
            nc.scalar.activation(sn_sb[:], ss_sb[:], func=AF.Sqrt,
                                 bias=eps_sb[:], scale=1.0)
            rn_sb = small.tile([K, 1], f32, tag="rn")
            nc.vector.reciprocal(rn_sb[:], sn_sb[:])
            if stage == 5.2:
                outT_sb = med.tile([K, D], f32, tag="outT")
                nc.vector.tensor_scalar_mul(outT_sb[:], vlad_sb[:], rn_sb[:])
                nc.sync.dma_start(out=out_d[i], in_=outT_sb[:])
                continue
            t2_sb = small.tile([K, 1], f32, tag="t2")
            nc.vector.tensor_scalar(out=t2_sb[:], in0=ss_sb[:],
                                    scalar1=rn_sb[:], scalar2=rn_sb[:],
                                    op0=OP.mult, op1=OP.mult)
            # global sumsq via partition reduction (ones matmul)
            nc.tensor.matmul(tiny_ps[0:1, 1:2], lhsT=onesf_sb[:, 0:1],
                             rhs=t2_sb[:], start=True, stop=True)
            gs_sb = small.tile([1, 1], f32, tag="gs")
            nc.scalar.activation(gs_sb[:], tiny_ps[0:1, 1:2], func=AF.Sqrt,
                                 bias=eps_sb[0:1, :], scale=1.0)
            rg_sb = small.tile([1, 1], f32, tag="rg")
            nc.vector.reciprocal(rg_sb[:], gs_sb[:])
            if stage == 5.3:
                outT_sb = med.tile([K, D], f32, tag="outT")
                nc.vector.tensor_scalar_mul(outT_sb[:], vlad_sb[:], rn_sb[:])
                nc.vector.tensor_copy(outT_sb[0:1, 0:1], rg_sb[:])
                nc.sync.dma_start(out=out_d[i], in_=outT_sb[:])
                continue
            # broadcast rg to 64 partitions
            nc.tensor.matmul(tiny_ps[:, 2:3], lhsT=onesf_sb[0:1, :],
                             rhs=rg_sb[:], start=True, stop=True)
            scale_sb = small.tile([K, 1], f32, tag="scale")
            nc.vector.tensor_mul(scale_sb[:], rn_sb[:], tiny_ps[:, 2:3])
            outT_sb = med.tile([K, D], f32, tag="outT")
            nc.vector.tensor_scalar_mul(outT_sb[:], vlad_sb[:], scale_sb[:])
            nc.sync.dma_start(out=out_d[i], in_=outT_sb[:])

    nc.compile()
    return nc


def _get_nc():
    if "nc" not in _CACHE:
        _CACHE["nc"] = _build()
    return _CACHE["nc"]


def _host_inputs(descriptors, W, b, centers):
    bf16 = ml_dtypes.bfloat16
    wt = np.ascontiguousarray(
        W.astype(np.float32).T.reshape(DT, 128, K)).astype(bf16)
    bias = np.ascontiguousarray(b.astype(np.float32).reshape(K, 1))
    cneg = np.ascontiguousarray((-centers.astype(np.float32).T))
    eye128 = np.eye(128, dtype=np.float32).astype(bf16)
    eye64 = np.eye(64, dtype=np.float32).astype(bf16)
    onesb = np.ones((128, 1), dtype=np.float32).astype(bf16)
    onesf = np.ones((64, 64), dtype=np.float32)
    common = {"wt": wt, "bias": bias, "cneg": cneg, "eye128": eye128,
              "eye64": eye64, "onesb": onesb, "onesf": onesf}
    in_maps = []
    for core in range(N_CORES):
        m = dict(common)
        m["desc"] = np.ascontiguousarray(
            descriptors[B_PER * core:B_PER * (core + 1)].astype(np.float32))
        in_maps.append(m)
    return in_maps


def _run(inputs, trace=False):
    from concourse.bass_utils import run_bass_kernel_spmd

    descriptors = np.asarray(inputs["descriptors"])
    W = np.asarray(inputs["W"])
    b = np.asarray(inputs["b"])
    centers = np.asarray(inputs["centers"])
    nc = _get_nc()
    in_maps = _host_inputs(descriptors, W, b, centers)
    res = run_bass_kernel_spmd(nc, in_maps, list(range(N_CORES)), trace=trace)
    outs = []
    for core in range(N_CORES):
        o = res.results[core]["out"]          # [B_PER, K, D]
        outs.append(np.transpose(o, (0, 2, 1)).reshape(B_PER, D * K))
    full = np.concatenate(outs, axis=0).astype(np.float32)
    return full, res


def kernel(**inputs):
    out, _ = _run(inputs, trace=False)
    return out


if __name__ == "__main__":
    rng = np.random.default_rng(0)
    inputs = {
        "descriptors": rng.standard_normal((B, D, N), dtype=np.float32),
        "W": (rng.standard_normal((K, D)) * 0.05).astype(np.float32),
        "b": (rng.standard_normal((K,)) * 0.05).astype(np.float32),
        "centers": rng.standard_normal((D, K)).astype(np.float32),
    }
    out = kernel(**inputs)
    print("out shape:", out.shape, out.dtype)



# revision 6
# speedup vs baseline: 1.4657x; 1.4657x over previous
"""NetVLAD layer on 8 Trainium2 NeuronCores (Bass/Tile), v2.

Problem: descriptors [B=16, D=512, N=4096] f32, W [K=64, D], b [K],
centers [D, K].
  scores = softmax_K(W @ desc + b)            [B, K, N]
  agg[b,d,k] = sum_n scores[b,k,n] desc[b,d,n]
  vlad = agg - centers * sum_n(scores);  intra-L2-norm over D; global L2.

Sharding: data-parallel over B across 8 cores (2 items per core);
W/b/centers replicated. No collectives.

v2 design (vs baseline):
  - host pre-casts desc to bf16 (halves HBM traffic; numerics unchanged)
  - mm1 in scoresT form: scoresT[n128,K] = desc_tc^T @ wt_t (full 128
    output partitions, softmax along the free dim, chunk-local)
  - bias folded as exp(b) multiplicative factor (Pool engine), not a
    PE rank-1 update: softTu = exp(scoresT) * ebb
  - 1/Z folded into the descT PSUM evacuation (per-partition scalar);
    ssum gets 1/Z as the moving matmul operand
  - mm2 in aggT form: aggT[K,D] += softTu_c^T @ (descT_c/Z), ssum[K,1]
    += softTu_c^T @ rzb_c  (shared stationary)
  - tail: after intra-norm the global sumsq is exactly K, so the final
    scale is rn = 1/(8*sqrt(ss)) = recip(sqrt(64*ss))
  - optional S_DMA: descT for the last S chunks comes pre-transposed
    from the host (DMA) instead of PE transposes
"""

import sys

sys.path.insert(0, "/opt/trn_rl_repo")

import numpy as np
import ml_dtypes

B, D, K, N = 16, 512, 64, 4096
N_CORES = 8
B_PER = B // N_CORES           # 2 items per core
DT = D // 128                  # 4 d-tiles
NCH = N // 128                 # 32 n-chunks of 128
S_DMA = 0                      # last S chunks: descT from host DMA

_CACHE = {}


def _build(stage=5, s_dma=S_DMA):
    import concourse.bass as bass  # noqa: F401
    import concourse.tile as tile
    from concourse import bacc, mybir
    from contextlib import ExitStack

    bf16 = mybir.dt.bfloat16
    f32 = mybir.dt.float32
    AF = mybir.ActivationFunctionType
    OP = mybir.AluOpType
    AX = mybir.AxisListType

    nc = bacc.Bacc("TRN2", target_bir_lowering=False, debug=False,
                   num_devices=N_CORES)

    desc_d = nc.dram_tensor("desc", [B_PER, DT, 128, N], bf16,
                            kind="ExternalInput").ap()
    wt_d = nc.dram_tensor("wt", [DT, 128, K], bf16, kind="ExternalInput").ap()
    ebb_d = nc.dram_tensor("ebb", [128, 8, K], bf16, kind="ExternalInput").ap()
    cnegT_d = nc.dram_tensor("cnegT", [K, D], f32, kind="ExternalInput").ap()
    eye_d = nc.dram_tensor("eye128", [128, 128], bf16,
                           kind="ExternalInput").ap()
    if s_dma:
        dTd_d = nc.dram_tensor("dTdma", [B_PER, s_dma, 128, D], bf16,
                               kind="ExternalInput").ap()
    out_d = nc.dram_tensor("out", [B_PER, K, D], f32, kind="ExternalOutput").ap()

    with tile.TileContext(nc) as tc, ExitStack() as ctx:
        const = ctx.enter_context(tc.tile_pool(name="const", bufs=1))
        descp = ctx.enter_context(tc.tile_pool(name="descp", bufs=2))
        grpp = ctx.enter_context(tc.tile_pool(name="grpp", bufs=2))
        dscp = ctx.enter_context(tc.tile_pool(name="dscp", bufs=6))
        small = ctx.enter_context(tc.tile_pool(name="small", bufs=4))
        tailp = ctx.enter_context(tc.tile_pool(name="tailp", bufs=2))
        if s_dma:
            dTdp = ctx.enter_context(tc.tile_pool(name="dTdp", bufs=2))
        ps_sc = ctx.enter_context(tc.tile_pool(name="ps_sc", bufs=2,
                                               space="PSUM"))
        ps_dT = ctx.enter_context(tc.tile_pool(name="ps_dT", bufs=4,
                                               space="PSUM"))
        ps_agg = ctx.enter_context(tc.tile_pool(name="ps_agg", bufs=1,
                                                space="PSUM"))
        ps_ss = ctx.enter_context(tc.tile_pool(name="ps_ss", bufs=1,
                                               space="PSUM"))

        # ---- constants ----
        wt_sb = const.tile([128, DT, K], bf16, tag="wt")
        for t in range(DT):
            nc.sync.dma_start(out=wt_sb[:, t, :], in_=wt_d[t])
        ebb_sb = const.tile([128, 8, K], bf16, tag="ebb")
        nc.sync.dma_start(out=ebb_sb[:], in_=ebb_d[:])
        cnegT_sb = const.tile([K, D], f32, tag="cnegT")
        nc.sync.dma_start(out=cnegT_sb[:], in_=cnegT_d[:])
        eye_sb = const.tile([128, 128], bf16, tag="eye")
        nc.sync.dma_start(out=eye_sb[:], in_=eye_d[:])
        onesb_sb = const.tile([128, 1], bf16, tag="onesb")
        nc.vector.memset(onesb_sb[:], 1.0)
        eps_sb = const.tile([K, 1], f32, tag="eps")
        nc.vector.memset(eps_sb[:], 1e-24)

        n_pe = NCH - s_dma  # chunks whose descT comes via PE transpose

        for i in range(B_PER):
            # ---- load descriptors (bf16, [128, DT, N]) ----
            desc_sb = descp.tile([128, DT, N], bf16, tag="desc")
            for half in range(2):
                nsl = slice(2048 * half, 2048 * (half + 1))
                for t in range(DT):
                    nc.sync.dma_start(out=desc_sb[:, t, nsl],
                                      in_=desc_d[i, t, :, nsl])
            if s_dma:
                dTd_sb = dTdp.tile([128, s_dma, D], bf16, tag="dTdma")
                nc.sync.dma_start(out=dTd_sb[:], in_=dTd_d[i])

            agg_ps = ps_agg.tile([K, D], f32, tag="agg")
            ss_ps = ps_ss.tile([K, 1], f32, tag="ss")

            # per-group state (group g = 8 chunks, one PSUM bank)
            scT = [None] * 4
            expT = [None] * 4
            softTu = [None] * 4
            rz = [None] * 4
            rzb = [None] * 4
            dsc = [None] * NCH      # evacuated descT tiles (or dma slices)
            softTs = [None] * NCH   # normalized soft tiles for DMA chunks

            evac_rr = [0]  # round-robin DVE/ACT for evacs
            dTpair = [None]  # current 2-chunk psum transpose tile

            def emit_mm1_half(h):
                g, hh = divmod(h, 2)
                if hh == 0:
                    scT[g] = ps_sc.tile([128, 8, K], f32, tag="scT", name="scT")
                for j4 in range(4):
                    j = 4 * hh + j4
                    c = 8 * g + j
                    csl = slice(128 * c, 128 * (c + 1))
                    pe_chunk = c < n_pe
                    if pe_chunk:
                        if c % 2 == 0:
                            dTpair[0] = ps_dT.tile([128, 2, DT, 128], bf16,
                                                   tag="dT", name="dT")
                        dT = dTpair[0]
                    for t in range(DT):
                        nc.tensor.matmul(
                            scT[g][:, j, :], lhsT=desc_sb[:, t, csl],
                            rhs=wt_sb[:, t, :],
                            start=(t == 0), stop=(t == DT - 1),
                        )
                        if pe_chunk:
                            nc.tensor.transpose(dT[:, c % 2, t, :],
                                                desc_sb[:, t, csl], eye_sb[:])
                    if pe_chunk:
                        dsc[c] = dT[:, c % 2, :, :]  # psum slice; evac later

            def emit_vec_half(h):
                g, hh = divmod(h, 2)
                jsl = slice(4 * hh, 4 * (hh + 1))
                if hh == 0:
                    expT[g] = grpp.tile([128, 8, K], bf16, tag="expT", name="expT")
                    softTu[g] = grpp.tile([128, 8, K], bf16, tag="softTu", name="softTu")
                    rz[g] = small.tile([128, 8], f32, tag="rz", name="rz")
                    rzb[g] = small.tile([128, 8], bf16, tag="rzb", name="rzb")
                z_sb = small.tile([128, 4], f32, tag="z")
                # exp on ACT (PSUM f32 -> SBUF bf16)
                nc.scalar.activation(out=expT[g][:, jsl, :],
                                     in_=scT[g][:, jsl, :], func=AF.Exp)
                # softTu = expT * exp(b)  (Pool, SBUF only)
                nc.gpsimd.tensor_tensor(out=softTu[g][:, jsl, :],
                                        in0=expT[g][:, jsl, :],
                                        in1=ebb_sb[:, jsl, :],
                                        op=OP.mult)
                # Z per chunk (Pool), then 1/Z (DVE) and bf16 copy (DVE)
                nc.vector.reduce_sum(z_sb[:], softTu[g][:, jsl, :], axis=AX.X)
                nc.vector.reciprocal(rz[g][:, jsl], z_sb[:])
                nc.gpsimd.tensor_copy(rzb[g][:, jsl], rz[g][:, jsl])
                if stage < 3:
                    return
                for j4 in range(4):
                    j = 4 * hh + j4
                    c = 8 * g + j
                    if c < n_pe:
                        # evac descT psum -> SBUF bf16, scaled by 1/Z
                        dT = dsc[c]
                        out_t = dscp.tile([128, DT, 128], bf16, tag="descTs", name="descTs")
                        if evac_rr[0] % 2 == 0:
                            nc.vector.tensor_scalar_mul(
                                out_t[:], dT, rz[g][:, j:j + 1])
                        else:
                            nc.scalar.mul(out_t[:], dT, rz[g][:, j:j + 1])
                        evac_rr[0] += 1
                        dsc[c] = out_t
                    else:
                        st = small.tile([128, K], bf16, tag="softTs", name="softTs")
                        nc.vector.tensor_scalar_mul(
                            st[:], softTu[g][:, j, :], rz[g][:, j:j + 1])
                        softTs[c] = st

            def emit_mm2_half(h):
                g, hh = divmod(h, 2)
                for j4 in range(4):
                    j = 4 * hh + j4
                    c = 8 * g + j
                    first = (c == 0)
                    last = (c == NCH - 1)
                    if c < n_pe:
                        lhs = softTu[g][:, j, :]
                        rhs = dsc[c][:]
                        srhs = rzb[g][:, j:j + 1]
                    else:
                        lhs = softTs[c][:]
                        rhs = dTd_sb[:, c - n_pe, :]
                        srhs = onesb_sb[:]
                    nc.tensor.matmul(agg_ps[:], lhsT=lhs, rhs=rhs,
                                     start=first, stop=last)
                    nc.tensor.matmul(ss_ps[:], lhsT=lhs, rhs=srhs,
                                     start=first, stop=last)

            # software-pipelined emission over 8 half-groups
            for h in range(8):
                emit_mm1_half(h)
                emit_vec_half(h)
                if stage >= 4 and h >= 2:
                    emit_mm2_half(h - 2)
            if stage >= 4:
                emit_mm2_half(6)
                emit_mm2_half(7)

            # debug-stage truncations (keep an output so nothing is DCE'd)
            if stage < 3:
                nc.sync.dma_start(out=out_d[i], in_=softTu[0][0:64, :, :])
                continue
            if stage < 4:
                nc.sync.dma_start(out=out_d[i], in_=dsc[0][0:64, :, :])
                continue

            # ---- tail ----
            ssum_sb = small.tile([K, 1], f32, tag="ssum")
            nc.scalar.copy(ssum_sb[:], ss_ps[:])
            vlad_sb = tailp.tile([K, D], f32, tag="vlad")
            nc.vector.scalar_tensor_tensor(
                vlad_sb[:], in0=cnegT_sb[:], scalar=ssum_sb[:], in1=agg_ps[:],
                op0=OP.mult, op1=OP.add,
            )
            if stage < 5:
                nc.sync.dma_start(out=out_d[i], in_=vlad_sb[:])
                continue
            sq_sb = tailp.tile([K, D], f32, tag="sq")
            nc.scalar.square(sq_sb[:], vlad_sb[:])
            ss2_sb = small.tile([K, 1], f32, tag="ss2")
            nc.vector.reduce_sum(ss2_sb[:], sq_sb[:], axis=AX.X)
            sn_sb = small.tile([K, 1], f32, tag="sn")
            # sqrt(64*ss + eps) = 8*sqrt(ss) ; global norm factor folded
            nc.scalar.activation(sn_sb[:], ss2_sb[:], func=AF.Sqrt,
                                 bias=eps_sb[:], scale=64.0)
            rn_sb = small.tile([K, 1], f32, tag="rn")
            nc.vector.reciprocal(rn_sb[:], sn_sb[:])
            outT_sb = tailp.tile([K, D], f32, tag="outT")
            nc.vector.tensor_scalar_mul(outT_sb[:], vlad_sb[:], rn_sb[:])
            nc.sync.dma_start(out=out_d[i], in_=outT_sb[:])

    nc.compile()
    return nc


def _get_nc():
    if "nc" not in _CACHE:
        _CACHE["nc"] = _build()
    return _CACHE["nc"]


def _host_inputs(descriptors, W, b, centers, s_dma=S_DMA):
    bf16 = ml_dtypes.bfloat16
    wt = np.ascontiguousarray(
        W.astype(np.float32).T.reshape(DT, 128, K)).astype(bf16)
    eb = np.exp(b.astype(np.float32)).astype(bf16).astype(np.float32)
    ebb = np.ascontiguousarray(
        np.broadcast_to(eb[None, None, :], (128, 8, K))).astype(bf16)
    cnegT = np.ascontiguousarray(-centers.astype(np.float32).T)
    eye = np.eye(128, dtype=np.float32).astype(bf16)
    desc_bf = descriptors.astype(bf16)  # [B, D, N]
    common = {"wt": wt, "ebb": ebb, "cnegT": cnegT, "eye128": eye}
    in_maps = []
    for core in range(N_CORES):
        m = dict(common)
        sl = desc_bf[B_PER * core:B_PER * (core + 1)]
        m["desc"] = np.ascontiguousarray(sl.reshape(B_PER, DT, 128, N))
        if s_dma:
            n0 = 128 * (NCH - s_dma)
            dT = sl[:, :, n0:]                      # [B_PER, D, s*128]
            dT = np.ascontiguousarray(dT.transpose(0, 2, 1))
            m["dTdma"] = dT.reshape(B_PER, s_dma, 128, D)
        in_maps.append(m)
    return in_maps


def _run(inputs, trace=False):
    from concourse.bass_utils import run_bass_kernel_spmd

    descriptors = np.asarray(inputs["descriptors"])
    W = np.asarray(inputs["W"])
    b = np.asarray(inputs["b"])
    centers = np.asarray(inputs["centers"])
    nc = _get_nc()
    in_maps = _host_inputs(descriptors, W, b, centers)
    res = run_bass_kernel_spmd(nc, in_maps, list(range(N_CORES)), trace=trace)
    outs = []
    for core in range(N_CORES):
        o = res.results[core]["out"]          # [B_PER, K, D]
        outs.append(np.transpose(o, (0, 2, 1)).reshape(B_PER, D * K))
    full = np.concatenate(outs, axis=0).astype(np.float32)
    return full, res


def kernel(**inputs):
    out, _ = _run(inputs, trace=False)
    return out


if __name__ == "__main__":
    rng = np.random.default_rng(0)
    inputs = {
        "descriptors": rng.standard_normal((B, D, N), dtype=np.float32),
        "W": (rng.standard_normal((K, D)) * 0.05).astype(np.float32),
        "b": (rng.standard_normal((K,)) * 0.05).astype(np.float32),
        "centers": rng.standard_normal((D, K)).astype(np.float32),
    }
    out = kernel(**inputs)
    print("out shape:", out.shape, out.dtype)


# revision 9
# speedup vs baseline: 1.6120x; 1.0998x over previous
"""NetVLAD layer on 8 Trainium2 NeuronCores (Bass/Tile), v2.

Problem: descriptors [B=16, D=512, N=4096] f32, W [K=64, D], b [K],
centers [D, K].
  scores = softmax_K(W @ desc + b)            [B, K, N]
  agg[b,d,k] = sum_n scores[b,k,n] desc[b,d,n]
  vlad = agg - centers * sum_n(scores);  intra-L2-norm over D; global L2.

Sharding: data-parallel over B across 8 cores (2 items per core);
W/b/centers replicated. No collectives.

v2 design (vs baseline):
  - host pre-casts desc to bf16 (halves HBM traffic; numerics unchanged)
  - mm1 in scoresT form: scoresT[n128,K] = desc_tc^T @ wt_t (full 128
    output partitions, softmax along the free dim, chunk-local)
  - bias folded as exp(b) multiplicative factor (Pool engine), not a
    PE rank-1 update: softTu = exp(scoresT) * ebb
  - 1/Z folded into the descT PSUM evacuation (per-partition scalar);
    ssum gets 1/Z as the moving matmul operand
  - mm2 in aggT form: aggT[K,D] += softTu_c^T @ (descT_c/Z), ssum[K,1]
    += softTu_c^T @ rzb_c  (shared stationary)
  - tail: after intra-norm the global sumsq is exactly K, so the final
    scale is rn = 1/(8*sqrt(ss)) = recip(sqrt(64*ss))
  - optional S_DMA: descT for the last S chunks comes pre-transposed
    from the host (DMA) instead of PE transposes
"""

import sys

sys.path.insert(0, "/opt/trn_rl_repo")

import numpy as np
import ml_dtypes

B, D, K, N = 16, 512, 64, 4096
N_CORES = 8
B_PER = B // N_CORES           # 2 items per core
DT = D // 128                  # 4 d-tiles
NCH = N // 128                 # 32 n-chunks of 128
S_DMA = 0                      # last S chunks: descT from host DMA

_CACHE = {}


def _build(stage=5, s_dma=S_DMA):
    import concourse.bass as bass  # noqa: F401
    import concourse.tile as tile
    from concourse import bacc, mybir
    from contextlib import ExitStack

    bf16 = mybir.dt.bfloat16
    f32 = mybir.dt.float32
    AF = mybir.ActivationFunctionType
    OP = mybir.AluOpType
    AX = mybir.AxisListType

    nc = bacc.Bacc("TRN2", target_bir_lowering=False, debug=False,
                   num_devices=N_CORES)

    desc_d = nc.dram_tensor("desc", [B_PER, DT, 128, N], bf16,
                            kind="ExternalInput").ap()
    wt_d = nc.dram_tensor("wt", [DT, 128, K], bf16, kind="ExternalInput").ap()
    ebb_d = nc.dram_tensor("ebb", [128, 8, K], bf16, kind="ExternalInput").ap()
    cnegT_d = nc.dram_tensor("cnegT", [K, D], f32, kind="ExternalInput").ap()
    eye_d = nc.dram_tensor("eye128", [128, 128], bf16,
                           kind="ExternalInput").ap()
    if s_dma:
        dTd_d = nc.dram_tensor("dTdma", [B_PER, s_dma, 128, D], bf16,
                               kind="ExternalInput").ap()
    out_d = nc.dram_tensor("out", [B_PER, K, D], f32, kind="ExternalOutput").ap()

    with tile.TileContext(nc) as tc, ExitStack() as ctx:
        const = ctx.enter_context(tc.tile_pool(name="const", bufs=1))
        descp = ctx.enter_context(tc.tile_pool(name="descp", bufs=2))
        grpp = ctx.enter_context(tc.tile_pool(name="grpp", bufs=2))
        dscp = ctx.enter_context(tc.tile_pool(name="dscp", bufs=6))
        small = ctx.enter_context(tc.tile_pool(name="small", bufs=4))
        stp = ctx.enter_context(tc.tile_pool(name="stp", bufs=14))
        tailp = ctx.enter_context(tc.tile_pool(name="tailp", bufs=2))
        if s_dma:
            dTdp = ctx.enter_context(tc.tile_pool(name="dTdp", bufs=2))
        ps_sc = ctx.enter_context(tc.tile_pool(name="ps_sc", bufs=2,
                                               space="PSUM"))
        ps_dT = ctx.enter_context(tc.tile_pool(name="ps_dT", bufs=4,
                                               space="PSUM"))
        ps_agg = ctx.enter_context(tc.tile_pool(name="ps_agg", bufs=1,
                                                space="PSUM"))
        ps_ss = ctx.enter_context(tc.tile_pool(name="ps_ss", bufs=1,
                                               space="PSUM"))

        # ---- constants (issued on the DVE HWDGE queue; desc goes on SP) ----
        wt_sb = const.tile([128, DT, K], bf16, tag="wt")
        for t in range(DT):
            nc.scalar.dma_start(out=wt_sb[:, t, :], in_=wt_d[t])
        eye_sb = const.tile([128, 128], bf16, tag="eye")
        nc.scalar.dma_start(out=eye_sb[:], in_=eye_d[:])
        ebb_sb = const.tile([128, 8, K], bf16, tag="ebb")
        nc.scalar.dma_start(out=ebb_sb[:], in_=ebb_d[:])
        cnegT_sb = const.tile([K, D], f32, tag="cnegT")
        nc.scalar.dma_start(out=cnegT_sb[:], in_=cnegT_d[:])
        onesb_sb = const.tile([128, 1], bf16, tag="onesb")
        nc.vector.memset(onesb_sb[:], 1.0)
        eps_sb = const.tile([K, 1], f32, tag="eps")
        nc.vector.memset(eps_sb[:], 1e-24)

        n_pe = NCH - s_dma  # chunks whose descT comes via PE transpose

        # ---- all input DMAs up front (before any output DMA lands on SP
        # queue, so item-1 loads are never stuck behind item-0's store) ----
        desc_sbs, dTd_sbs = [], []
        for i in range(B_PER):
            desc_sb = descp.tile([128, DT, N], bf16, tag="desc", name="desc_sb")
            if i == 0:
                # finer first pieces so group 0 can start ASAP
                for q in range(2):
                    nsl = slice(1024 * q, 1024 * (q + 1))
                    for t in range(DT):
                        nc.sync.dma_start(out=desc_sb[:, t, nsl],
                                          in_=desc_d[i, t, :, nsl])
                nsl = slice(2048, 4096)
                for t in range(DT):
                    nc.sync.dma_start(out=desc_sb[:, t, nsl],
                                      in_=desc_d[i, t, :, nsl])
            else:
                for half in range(2):
                    nsl = slice(2048 * half, 2048 * (half + 1))
                    for t in range(DT):
                        nc.sync.dma_start(out=desc_sb[:, t, nsl],
                                          in_=desc_d[i, t, :, nsl])
            desc_sbs.append(desc_sb)
            if s_dma:
                dTd_sb = dTdp.tile([128, s_dma, D], bf16, tag="dTdma",
                                   name="dTd_sb")
                nc.sync.dma_start(out=dTd_sb[:], in_=dTd_d[i])
                dTd_sbs.append(dTd_sb)

        for i in range(B_PER):
            desc_sb = desc_sbs[i]
            if s_dma:
                dTd_sb = dTd_sbs[i]

            agg_ps = ps_agg.tile([K, D], f32, tag="agg")
            ss_ps = ps_ss.tile([K, 1], f32, tag="ss")

            # per-group state (group g = 8 chunks, one PSUM bank)
            scT = [None] * 4
            expT = [None] * 4
            softTu = [None] * 4
            rz = [None] * 4
            rzb = [None] * 4
            dsc = [None] * NCH      # evacuated descT tiles (or dma slices)
            softTs = [None] * NCH   # normalized soft tiles for DMA chunks

            evac_rr = [0]  # round-robin DVE/ACT for evacs
            dTpair = [None]  # current 2-chunk psum transpose tile

            def emit_mm1_half(h):
                g, hh = divmod(h, 2)
                if hh == 0:
                    scT[g] = ps_sc.tile([128, 8, K], f32, tag="scT", name="scT")
                for j4 in range(4):
                    j = 4 * hh + j4
                    c = 8 * g + j
                    csl = slice(128 * c, 128 * (c + 1))
                    pe_chunk = c < n_pe
                    if pe_chunk:
                        if c % 2 == 0:
                            dTpair[0] = ps_dT.tile([128, 2, DT, 128], bf16,
                                                   tag="dT", name="dT")
                        dT = dTpair[0]
                    for t in range(DT):
                        nc.tensor.matmul(
                            scT[g][:, j, :], lhsT=desc_sb[:, t, csl],
                            rhs=wt_sb[:, t, :],
                            start=(t == 0), stop=(t == DT - 1),
                        )
                        if pe_chunk:
                            nc.tensor.transpose(dT[:, c % 2, t, :],
                                                desc_sb[:, t, csl], eye_sb[:])
                    if pe_chunk:
                        dsc[c] = dT[:, c % 2, :, :]  # psum slice; evac later

            def emit_vec_half(h):
                g, hh = divmod(h, 2)
                jsl = slice(4 * hh, 4 * (hh + 1))
                if hh == 0:
                    expT[g] = grpp.tile([128, 8, K], bf16, tag="expT", name="expT")
                    softTu[g] = grpp.tile([128, 8, K], bf16, tag="softTu", name="softTu")
                    rz[g] = small.tile([128, 8], f32, tag="rz", name="rz")
                    rzb[g] = small.tile([128, 8], bf16, tag="rzb", name="rzb")
                z_sb = small.tile([128, 4], f32, tag="z")
                # exp on ACT (PSUM f32 -> SBUF bf16)
                nc.scalar.activation(out=expT[g][:, jsl, :],
                                     in_=scT[g][:, jsl, :], func=AF.Exp)
                # softTu = expT * exp(b)  (Pool, SBUF only)
                nc.gpsimd.tensor_tensor(out=softTu[g][:, jsl, :],
                                        in0=expT[g][:, jsl, :],
                                        in1=ebb_sb[:, jsl, :],
                                        op=OP.mult)
                # Z per chunk (Pool), then 1/Z (DVE) and bf16 copy (DVE)
                nc.vector.reduce_sum(z_sb[:], softTu[g][:, jsl, :], axis=AX.X)
                nc.vector.reciprocal(rz[g][:, jsl], z_sb[:])
                nc.gpsimd.tensor_copy(rzb[g][:, jsl], rz[g][:, jsl])
                if stage < 3:
                    return
                for j4 in range(4):
                    j = 4 * hh + j4
                    c = 8 * g + j
                    if c < n_pe:
                        # evac descT psum -> SBUF bf16, scaled by 1/Z
                        dT = dsc[c]
                        out_t = dscp.tile([128, DT, 128], bf16, tag="descTs", name="descTs")
                        if evac_rr[0] % 3 < 2:
                            nc.vector.tensor_scalar_mul(
                                out_t[:], dT, rz[g][:, j:j + 1])
                        else:
                            nc.scalar.mul(out_t[:], dT, rz[g][:, j:j + 1])
                        evac_rr[0] += 1
                        dsc[c] = out_t
                    else:
                        st = stp.tile([128, K], bf16, tag="softTs", name="softTs")
                        nc.gpsimd.tensor_scalar_mul(
                            st[:], softTu[g][:, j, :], rz[g][:, j:j + 1])
                        softTs[c] = st

            def emit_mm2_half(h):
                g, hh = divmod(h, 2)
                for j4 in range(4):
                    j = 4 * hh + j4
                    c = 8 * g + j
                    first = (c == 0)
                    last = (c == NCH - 1)
                    if c < n_pe:
                        lhs = softTu[g][:, j, :]
                        rhs = dsc[c][:]
                        srhs = rzb[g][:, j:j + 1]
                    else:
                        lhs = softTs[c][:]
                        rhs = dTd_sb[:, c - n_pe, :]
                        srhs = onesb_sb[:]
                    nc.tensor.matmul(agg_ps[:], lhsT=lhs, rhs=rhs,
                                     start=first, stop=last)
                    nc.tensor.matmul(ss_ps[:], lhsT=lhs, rhs=srhs,
                                     start=first, stop=last)

            # software-pipelined emission over 8 half-groups
            for h in range(8):
                emit_mm1_half(h)
                emit_vec_half(h)
                if stage >= 4 and h >= 2:
                    emit_mm2_half(h - 2)
            if stage >= 4:
                emit_mm2_half(6)
                emit_mm2_half(7)

            # debug-stage truncations (keep an output so nothing is DCE'd)
            if stage < 3:
                nc.sync.dma_start(out=out_d[i], in_=softTu[0][0:64, :, :])
                continue
            if stage < 4:
                nc.sync.dma_start(out=out_d[i], in_=dsc[0][0:64, :, :])
                continue

            # ---- tail ----
            ssum_sb = small.tile([K, 1], f32, tag="ssum")
            nc.vector.tensor_copy(ssum_sb[:], ss_ps[:])
            vlad_sb = tailp.tile([K, D], f32, tag="vlad")
            nc.vector.scalar_tensor_tensor(
                vlad_sb[:], in0=cnegT_sb[:], scalar=ssum_sb[:], in1=agg_ps[:],
                op0=OP.mult, op1=OP.add,
            )
            if stage < 5:
                nc.sync.dma_start(out=out_d[i], in_=vlad_sb[:])
                continue
            sq_sb = tailp.tile([K, D], f32, tag="sq")
            nc.gpsimd.tensor_mul(sq_sb[:], vlad_sb[:], vlad_sb[:])
            ss2_sb = small.tile([K, 1], f32, tag="ss2")
            nc.vector.reduce_sum(ss2_sb[:], sq_sb[:], axis=AX.X)
            sn_sb = small.tile([K, 1], f32, tag="sn")
            # sqrt(64*ss + eps) = 8*sqrt(ss) ; global norm factor folded
            nc.scalar.activation(sn_sb[:], ss2_sb[:], func=AF.Sqrt,
                                 bias=eps_sb[:], scale=64.0)
            rn_sb = small.tile([K, 1], f32, tag="rn")
            nc.vector.reciprocal(rn_sb[:], sn_sb[:])
            outT_sb = tailp.tile([K, D], f32, tag="outT")
            nc.vector.tensor_scalar_mul(outT_sb[:], vlad_sb[:], rn_sb[:])
            nc.sync.dma_start(out=out_d[i], in_=outT_sb[:])

    nc.compile()
    return nc


def _get_nc():
    if "nc" not in _CACHE:
        _CACHE["nc"] = _build()
    return _CACHE["nc"]


def _host_inputs(descriptors, W, b, centers, s_dma=S_DMA):
    bf16 = ml_dtypes.bfloat16
    wt = np.ascontiguousarray(
        W.astype(np.float32).T.reshape(DT, 128, K)).astype(bf16)
    eb = np.exp(b.astype(np.float32)).astype(bf16).astype(np.float32)
    ebb = np.ascontiguousarray(
        np.broadcast_to(eb[None, None, :], (128, 8, K))).astype(bf16)
    cnegT = np.ascontiguousarray(-centers.astype(np.float32).T)
    eye = np.eye(128, dtype=np.float32).astype(bf16)
    desc_bf = descriptors.astype(bf16)  # [B, D, N]
    common = {"wt": wt, "ebb": ebb, "cnegT": cnegT, "eye128": eye}
    in_maps = []
    for core in range(N_CORES):
        m = dict(common)
        sl = desc_bf[B_PER * core:B_PER * (core + 1)]
        m["desc"] = np.ascontiguousarray(sl.reshape(B_PER, DT, 128, N))
        if s_dma:
            n0 = 128 * (NCH - s_dma)
            dT = sl[:, :, n0:]                      # [B_PER, D, s*128]
            dT = np.ascontiguousarray(dT.transpose(0, 2, 1))
            m["dTdma"] = dT.reshape(B_PER, s_dma, 128, D)
        in_maps.append(m)
    return in_maps


def _run(inputs, trace=False):
    from concourse.bass_utils import run_bass_kernel_spmd

    descriptors = np.asarray(inputs["descriptors"])
    W = np.asarray(inputs["W"])
    b = np.asarray(inputs["b"])
    centers = np.asarray(inputs["centers"])
    nc = _get_nc()
    in_maps = _host_inputs(descriptors, W, b, centers)
    res = run_bass_kernel_spmd(nc, in_maps, list(range(N_CORES)), trace=trace)
    outs = []
    for core in range(N_CORES):
        o = res.results[core]["out"]          # [B_PER, K, D]
        outs.append(np.transpose(o, (0, 2, 1)).reshape(B_PER, D * K))
    full = np.concatenate(outs, axis=0).astype(np.float32)
    return full, res


def kernel(**inputs):
    out, _ = _run(inputs, trace=False)
    return out


if __name__ == "__main__":
    rng = np.random.default_rng(0)
    inputs = {
        "descriptors": rng.standard_normal((B, D, N), dtype=np.float32),
        "W": (rng.standard_normal((K, D)) * 0.05).astype(np.float32),
        "b": (rng.standard_normal((K,)) * 0.05).astype(np.float32),
        "centers": rng.standard_normal((D, K)).astype(np.float32),
    }
    out = kernel(**inputs)
    print("out shape:", out.shape, out.dtype)
